# revision 1
# baseline (speedup 1.0000x reference)
"""DAML dense_cnn Trainium2 Bass kernel.

Data-parallel over batch: B=64 -> 8 NeuronCores x 8 batches each.

Per-core pipeline (per side u/i):
  1. dma_gather (transpose mode, bf16) pulls e^T = emb[doc]^T directly into
     SBUF as (128 dpart, 3 chunks, 1024 tok-cols) per 2-batch group. The
     50k-vocab int16-index limit is beaten by biasing the table base by
     32768 rows and feeding idx-32768 (the ucode sign-extends, so negative
     offsets address the low half). Pad/guard positions gather a
     host-appended all-zero row (id 50000), giving exact zero conv padding.
  2. gate: 3 matmuls (K=128 chunks, M=3 taps, N=502) -> psum u(3,502);
     2 DVE adds combine the taps with column shifts; ACT Sigmoid(+wcb bias)
     -> g (1,500); gpsimd partition_broadcast -> (128,500); DVE mult gates
     e^T in place.
  3. conv: 9 matmuls (3 taps x 3 chunks) accumulate feat psum (100,500)
     via shifted rhs windows.  feat -> sbuf (102,500) bf16 (+dc_b bias,
     user side scaled by -2).  Aug rows: row {100|101} = sum_f feat^2
     (user x0.25), other = ones.  K=102 einsum then yields
     sq[l,m] = |u_l - v_m|^2 directly in psum.
  4. att = sigmoid(-0.5*ln(sq)) == 1/(1+sqrt(sq)).  ACT Ln then ACT Sigmoid
     with accum_out giving user row-sums free; item col-sums via
     ones-matmuls on att tiles.  Chunked so ACT table loads stay rare.
  5. Pooling collapses to S_k[f] = sum_l c_k[l]*att_sum[l]*feat[f,l]
     (c_k = sliding-window counts): PE transposes of feat + small matmuls,
     abs-conv contraction (bias via aug row), fc matmul, ACT relu(+bias),
     PE transpose, id-embedding indirect gather, DMA out.
"""
import os
import numpy as np
import ml_dtypes

import concourse.bass as bass
import concourse.bacc as bacc
import concourse.tile as tile
from concourse import mybir
from concourse import bass_utils

BF16 = ml_dtypes.bfloat16
DT = mybir.dt
AF = mybir.ActivationFunctionType
ALU = mybir.AluOpType

B, L, V, D, F, ID = 64, 500, 50000, 300, 100, 32
NCORE = 8
BLOC = B // NCORE            # batches per core
DPAD = 384                   # D padded to 3*128
NCH = 3
PADROW = V                   # zero row appended to tables
BIAS = 32768                 # int16 index bias
GW = 512                     # tokens per gather group (1 batch)
NG = BLOC                    # gather groups per side
SEG = 502                    # batch segment stride inside a group
LT = [128, 128, 128, 116]
LT_OFF = [0, 128, 256, 384]
ATT_CHUNK = int(os.environ.get("DAML_ATT_CHUNK", "8"))
STAGE = int(os.environ.get("DAML_STAGE", "4"))
NO_ACCUM = os.environ.get("DAML_NO_ACCUM", "1") == "1"
NO_IATT = os.environ.get("DAML_NO_IATT", "0") == "1"
NO_LNBIAS = os.environ.get("DAML_NO_LNBIAS", "0") == "1"
LOOP = int(os.environ.get("DAML_LOOP", "1"))
GATE_CT = os.environ.get("DAML_GATE_CT", "0") == "1"


def build_program():
    nc = bacc.Bacc("TRN2", target_bir_lowering=False, debug=False,
                   num_devices=NCORE)
    t = {}

    def din(name, shape, dt):
        t[name] = nc.dram_tensor(name, shape, dt, kind="ExternalInput")

    for s in "ui":
        din(f"tab_{s}", (V + 1, DPAD), DT.bfloat16)
        din(f"idx_{s}", (128, 32 * NG), DT.int16)
        din(f"wd_{s}", (128, NCH, 3, F), DT.bfloat16)
        din(f"dcb_{s}", (F, 1), DT.float32)
        din(f"wabs_{s}", (101, 3, F), DT.bfloat16)
        din(f"wfc_{s}", (F, ID), DT.bfloat16)
        din(f"bfc_{s}", (ID, 1), DT.float32)
        din(f"idemb_{s}", (10002, ID), DT.float32)
        din(f"idids_{s}", (BLOC, 1), DT.int32)
    din("w3", (128, NCH, 3), DT.bfloat16)
    din("wcb", (1, 1), DT.float32)
    din("ck", (128, 4, 3), DT.bfloat16)
    din("ones_row", (1, L), DT.bfloat16)
    t["v2s_scratch"] = nc.dram_tensor("v2s_scratch", (BLOC, L), DT.bfloat16,
                                      kind="Internal")
    t["dbg"] = nc.dram_tensor("dbg", (128, 3 * GW), DT.float32,
                              kind="ExternalOutput")
    t["out_use"] = nc.dram_tensor("out_use", (BLOC, 2 * ID), DT.float32,
                                  kind="ExternalOutput")
    t["out_item"] = nc.dram_tensor("out_item", (BLOC, 2 * ID), DT.float32,
                                   kind="ExternalOutput")

    with tile.TileContext(nc) as tc:
        _emit(nc, tc, t)

    nc.compile()
    return nc


def _emit(nc, tc, t):
    from contextlib import ExitStack
    from concourse.masks import make_identity
    ctx = ExitStack()

    consts = ctx.enter_context(tc.tile_pool(name="consts", bufs=1))
    et_pool = ctx.enter_context(tc.tile_pool(name="et", bufs=1))
    feat_pool = ctx.enter_context(tc.tile_pool(name="feat", bufs=1))
    ln_pool = ctx.enter_context(tc.tile_pool(name="ln", bufs=4 * ATT_CHUNK + 2))
    att_pool = ctx.enter_context(tc.tile_pool(name="att", bufs=6))
    sm_pool = ctx.enter_context(tc.tile_pool(name="sm", bufs=8))
    gbc_pool = ctx.enter_context(tc.tile_pool(name="gbc", bufs=6))
    u2_pool = ctx.enter_context(tc.tile_pool(name="u2", bufs=4))
    ft_pool = ctx.enter_context(tc.tile_pool(name="ft", bufs=6))

    psum = ctx.enter_context(tc.tile_pool(name="psum", bufs=2, space="PSUM"))
    ps_S = ctx.enter_context(tc.tile_pool(name="ps_S", bufs=1, space="PSUM"))

    # ---------------- constants / weights ----------------
    # order matters: idx + gate weights first so gathers + gate start early
    idx_sb, wd_sb, wabs_sb, wfc_sb, bfc_sb, dcb_sb, idid_sb = ({} for _ in range(7))
    for s in "ui":
        idx_sb[s] = consts.tile([128, 32 * NG], DT.int16, tag=f"idx{s}", name=f"idx{s}")
        nc.sync.dma_start(out=idx_sb[s][:], in_=t[f"idx_{s}"].ap())
    w3_sb = consts.tile([128, NCH, 3], DT.bfloat16, tag="w3", name="w3")
    nc.sync.dma_start(out=w3_sb[:], in_=t["w3"].ap())
    wcb_sb = consts.tile([1, 1], DT.float32, tag="wcb", name="wcb")
    nc.sync.dma_start(out=wcb_sb[:], in_=t["wcb"].ap())
    for s in "ui":
        wd_sb[s] = consts.tile([128, NCH, 3, F], DT.bfloat16, tag=f"wd{s}", name=f"wd{s}")
        nc.sync.dma_start(out=wd_sb[s][:], in_=t[f"wd_{s}"].ap())
        dcb_sb[s] = consts.tile([F, 1], DT.float32, tag=f"dcb{s}", name=f"dcb{s}")
        nc.sync.dma_start(out=dcb_sb[s][:], in_=t[f"dcb_{s}"].ap())
    for s in "ui":
        wabs_sb[s] = consts.tile([101, 3, F], DT.bfloat16, tag=f"wabs{s}", name=f"wabs{s}")
        nc.sync.dma_start(out=wabs_sb[s][:], in_=t[f"wabs_{s}"].ap())
        wfc_sb[s] = consts.tile([F, ID], DT.bfloat16, tag=f"wfc{s}", name=f"wfc{s}")
        nc.sync.dma_start(out=wfc_sb[s][:], in_=t[f"wfc_{s}"].ap())
        bfc_sb[s] = consts.tile([ID, 1], DT.float32, tag=f"bfc{s}", name=f"bfc{s}")
        nc.sync.dma_start(out=bfc_sb[s][:], in_=t[f"bfc_{s}"].ap())
        idid_sb[s] = consts.tile([BLOC, 1], DT.int32, tag=f"idid{s}", name=f"idid{s}")
        nc.sync.dma_start(out=idid_sb[s][:], in_=t[f"idids_{s}"].ap())
    ck_sb = consts.tile([128, 4, 3], DT.bfloat16, tag="ck", name="ck")
    nc.sync.dma_start(out=ck_sb[:], in_=t["ck"].ap())

    ones_bf = consts.tile([128, 1], DT.bfloat16, tag="ones", name="ones")
    nc.vector.memset(ones_bf[:], 1.0)
    ident_bf = consts.tile([128, 128], DT.bfloat16, tag="identb", name="identb")
    make_identity(nc, ident_bf[:])
    ident_f32 = consts.tile([ID, ID], DT.float32, tag="identf", name="identf")
    make_identity(nc, ident_f32[:])

    if LOOP > 1:
        ctx.enter_context(tc.For_i(0, LOOP, 1))

    # ---------------- gathers (per 2-batch group) ----------------
    eT = {}
    for s in "ui":
        for g in range(NG):
            eT[(s, g)] = et_pool.tile([128, NCH, GW], DT.bfloat16, tag=f"eT{s}{g}", name=f"eT{s}{g}")
            nc.gpsimd.dma_gather(
                out_ap=eT[(s, g)][:],
                in_ap=t[f"tab_{s}"].ap()[BIAS:, :],
                idxs_ap=idx_sb[s][:, 32 * g:32 * (g + 1)],
                num_idxs=GW, num_idxs_reg=GW,
                elem_size=DPAD, transpose=True,
            )

    if STAGE == 1:
        dbgt = consts.tile([128, NCH, GW], DT.float32, tag="dbgt", name="dbgt")
        nc.vector.tensor_copy(dbgt[:], eT[("u", 0)][:])
        nc.sync.dma_start(out=t["dbg"].ap(), in_=dbgt[:].rearrange("p c t -> p (c t)"))
        ctx.close()
        return

    feat_sb = {(s, b): feat_pool.tile([101, L], DT.bfloat16, tag=f"feat{s}{b}", name=f"feat{s}{b}")
               for s in "ui" for b in range(BLOC)}
    uattT = consts.tile([128, 4, BLOC], DT.float32, tag="uatt", name="uatt")
    iattT = {}

    # ---------------- phase 1: gate -> eg -> conv -> feat (sw-pipelined) ----
    bs_list = [(s, b) for b in range(BLOC) for s in "ui"]
    g_bc = {}
    u2sT_b = {}

    def gate_block(s, b):
        g, base = b, 0
        if GATE_CT:
            # taps col-tiled to partitions 0/32/64 (concurrent on PE),
            # then 3 row-tiled K=1 matmuls re-sum with the tap shifts.
            ups = psum.tile([128, SEG], DT.float32, tag="u", name="u")
            for k in range(3):
                for c in range(NCH):
                    nc.tensor.matmul(out=ups[32 * k:32 * k + 1, :],
                                     lhsT=w3_sb[:, c, k:k + 1],
                                     rhs=eT[(s, g)][:, c, base:base + SEG],
                                     start=(c == 0), stop=(c == NCH - 1),
                                     tile_position=(0, 32 * k))
            u_sb = sm_pool.tile([65, SEG], DT.bfloat16, tag="u_sb", name="u_sb")
            nc.vector.tensor_copy(u_sb[:, :], ups[0:65, :])
            gp = psum.tile([1, L], DT.float32, tag="u", name="gp")
            for k in range(3):
                nc.tensor.matmul(out=gp[:, :],
                                 lhsT=ones_bf[32 * k:32 * k + 1, :],
                                 rhs=u_sb[32 * k:32 * k + 1, k:k + L],
                                 start=(k == 0), stop=(k == 2),
                                 tile_position=(32 * k, 0))
            gsrc = gp
        else:
            ups = psum.tile([1, L], DT.float32, tag="u", name="u")
            # 9 matmuls, shifted rhs windows, all accumulate one psum row
            k = 0
            for tp in range(3):
                for c in range(NCH):
                    nc.tensor.matmul(out=ups[:, :],
                                     lhsT=w3_sb[:, c, tp:tp + 1],
                                     rhs=eT[(s, g)][:, c, base + tp:base + tp + L],
                                     start=(k == 0), stop=(k == 8))
                    k += 1
            gsrc = ups
        g_sb = sm_pool.tile([1, L], DT.bfloat16, tag="g_sb", name="g_sb")
        nc.scalar.activation(g_sb[:, :], gsrc[:, :], AF.Sigmoid,
                             bias=wcb_sb[:, :])
        gb = gbc_pool.tile([128, L], DT.bfloat16, tag="g_bc", name="g_bc")
        nc.gpsimd.partition_broadcast(gb[:, :], g_sb[:, :])
        g_bc[(s, b)] = gb

    def conv_block(s, b):
        g, base = b, 0
        tok = base + 1
        gb = g_bc[(s, b)]
        nc.vector.tensor_tensor(out=eT[(s, g)][:, :, tok:tok + L],
                                in0=eT[(s, g)][:, :, tok:tok + L],
                                in1=gb[:, None, :].to_broadcast([128, NCH, L]),
                                op=ALU.mult)
        fps = psum.tile([F, L], DT.float32, tag="feat", name="feat")
        k = 0
        for tp in range(3):
            for c in range(NCH):
                nc.tensor.matmul(out=fps[:, :], lhsT=wd_sb[s][:, c, tp, :],
                                 rhs=eT[(s, g)][:, c, base + tp:base + tp + L],
                                 start=(k == 0), stop=(k == 8))
                k += 1
        fsb = feat_sb[(s, b)]
        nc.vector.tensor_scalar(out=fsb[0:F, :], in0=fps[:, :],
                                scalar1=(-2.0 if s == "u" else 1.0),
                                scalar2=dcb_sb[s][:, :],
                                op0=ALU.mult, op1=ALU.add)
        u2 = u2_pool.tile([F, L], DT.bfloat16, tag="u2", name="u2")
        nc.vector.tensor_tensor(out=u2[:, :], in0=fsb[0:F, :],
                                in1=fsb[0:F, :], op=ALU.mult)
        if s == "u":
            # row 100 = ones (DMA: engine writes must be 32-aligned)
            nc.sync.dma_start(out=fsb[100:101, :], in_=t["ones_row"].ap())
            # u2sT[l] = 0.25 * sum_f (-2 feat)^2, l on partitions (ln bias)
            ut = psum.tile([128, 4], DT.float32, tag="u", name="ut")
            for lt in range(4):
                m = LT[lt]
                nc.tensor.matmul(out=ut[0:m, lt:lt + 1],
                                 lhsT=u2[:, LT_OFF[lt]:LT_OFF[lt] + m],
                                 rhs=ones_bf[0:F, :], start=True, stop=True)
            u2sT = consts.tile([128, 4], DT.float32, tag=f"u2sT{b}",
                               name=f"u2sT{b}")
            nc.vector.tensor_scalar(out=u2sT[:, :], in0=ut[:, :],
                                    scalar1=0.25, scalar2=None, op0=ALU.mult)
            u2sT_b[b] = u2sT
        else:
            # row 100 = v2s = sum_f feat^2 (stage via partition 0 + DMA)
            v2s = psum.tile([1, L], DT.float32, tag="u", name="v2s")
            nc.tensor.matmul(out=v2s[:, :], lhsT=ones_bf[0:F, :], rhs=u2[:, :],
                             start=True, stop=True)
            v2st = sm_pool.tile([1, L], DT.bfloat16, tag="v2st", name="v2st")
            nc.vector.tensor_copy(v2st[:, :], v2s[:, :])
            # bounce via DRAM: SBUF->SBUF DMA deadlocks vs xbar gathers
            nc.sync.dma_start(out=t["v2s_scratch"].ap()[b:b + 1, :],
                              in_=v2st[:, :])
            nc.sync.dma_start(out=fsb[100:101, :],
                              in_=t["v2s_scratch"].ap()[b:b + 1, :])

    DEPTH = 2
    for i, (s, b) in enumerate(bs_list):
        gate_block(s, b)
        if i >= DEPTH:
            conv_block(*bs_list[i - DEPTH])
    for i in range(len(bs_list) - DEPTH, len(bs_list)):
        conv_block(*bs_list[i])

    if STAGE == 2:
        dbgt = consts.tile([128, 2 * L], DT.float32, tag="dbgt", name="dbgt")
        nc.vector.tensor_copy(dbgt[0:101, 0:L], feat_sb[("u", 0)][:])
        nc.vector.tensor_copy(dbgt[0:101, L:2 * L], feat_sb[("i", 0)][:])
        nc.sync.dma_start(out=t["dbg"].ap()[:, 0:2 * L], in_=dbgt[:])
        ctx.close()
        return

    # ---------------- phase 2/3: einsum -> ln -> sigmoid -> att sums -------
    for chunk0 in range(0, BLOC, ATT_CHUNK):
        batches = list(range(chunk0, min(chunk0 + ATT_CHUNK, BLOC)))
        ln_tiles = {}
        for b in batches:
            for lt in range(4):
                m = LT[lt]
                sq = psum.tile([128, L], DT.float32, tag="sq", name="sq")
                nc.tensor.matmul(
                    out=sq[:m, :],
                    lhsT=feat_sb[("u", b)][:, LT_OFF[lt]:LT_OFF[lt] + m],
                    rhs=feat_sb[("i", b)][:, :],
                    start=True, stop=True)
                lnt = ln_pool.tile([128, L], DT.bfloat16, tag="lnt", name="lnt")
                if NO_LNBIAS:
                    nc.scalar.activation(lnt[:m, :], sq[:m, :], AF.Ln)
                else:
                    nc.scalar.activation(lnt[:m, :], sq[:m, :], AF.Ln,
                                         bias=u2sT_b[b][0:m, lt:lt + 1])
                ln_tiles[(b, lt)] = lnt
        for b in batches:
            ia = psum.tile([128, 4], DT.float32, tag="feat", name="feat")
            atts = {}
            for lt in range(4):
                m = LT[lt]
                att = att_pool.tile([128, L], DT.bfloat16, tag="att", name="att")
                atts[lt] = att
                if NO_ACCUM:
                    nc.scalar.activation(att[:m, :], ln_tiles[(b, lt)][:m, :],
                                         AF.Sigmoid, scale=-0.5)
                    nc.vector.tensor_reduce(out=uattT[:m, lt, b:b + 1],
                                            in_=att[:m, :],
                                            axis=mybir.AxisListType.X,
                                            op=ALU.add)
                else:
                    nc.scalar.activation(att[:m, :], ln_tiles[(b, lt)][:m, :],
                                         AF.Sigmoid, scale=-0.5,
                                         accum_out=uattT[:m, lt, b:b + 1])
            # sum the three 128-row att tiles on DVE, then column-sum the
            # sum tile (K=128) and the 116-row tail tile (K=116) on PE.
            # ms outer / lt-group inner keeps each psum column's
            # accumulation group contiguous (start clears has_written
            # beyond the written AP).
            asum = att_pool.tile([128, L], DT.bfloat16, tag="asum", name="asum")
            nc.vector.tensor_tensor(out=asum[:, :], in0=atts[0][:, :],
                                    in1=atts[1][:, :], op=ALU.add)
            nc.vector.tensor_tensor(out=asum[:, :], in0=asum[:, :],
                                    in1=atts[2][:, :], op=ALU.add)
            for ms in range(4):
                if NO_IATT:
                    break
                nc.tensor.matmul(
                    out=ia[0:LT[ms], ms:ms + 1],
                    lhsT=asum[:, LT_OFF[ms]:LT_OFF[ms] + LT[ms]],
                    rhs=ones_bf[:, :], start=True, stop=False)
                nc.tensor.matmul(
                    out=ia[0:LT[ms], ms:ms + 1],
                    lhsT=atts[3][:116, LT_OFF[ms]:LT_OFF[ms] + LT[ms]],
                    rhs=ones_bf[:116, :], start=False, stop=True)
            iab = consts.tile([128, 4], DT.float32, tag=f"iatt{b}", name=f"iatt{b}")
            nc.vector.tensor_copy(iab[:, :], ia[:, :])
            iattT[b] = iab

    if STAGE == 3:
        dbgt = consts.tile([128, 4 * BLOC + 4 * BLOC], DT.float32, tag="dbgt", name="dbgt")
        for lt in range(4):
            nc.vector.tensor_copy(dbgt[:, BLOC * lt:BLOC * (lt + 1)], uattT[:, lt, :])
        for b in range(BLOC):
            nc.vector.tensor_copy(dbgt[:, 4 * BLOC + 4 * b:4 * BLOC + 4 * (b + 1)],
                                  iattT[b][:])
        nc.sync.dma_start(out=t["dbg"].ap()[:, 0:8 * BLOC], in_=dbgt[:])
        ctx.close()
        return

    # ---------------- phase 4: pooling ----------------
    S_ps = ps_S.tile([F, 6 * BLOC], DT.float32, tag="S", name="S")
    for si, s in enumerate("ui"):
        for b in range(BLOC):
            col = 3 * (BLOC * si + b)
            wa = sm_pool.tile([128, 4, 3], DT.bfloat16, tag="wa", name="wa")
            attsrc = uattT[:, :, b:b + 1] if s == "u" else iattT[b][:, :, None]
            nc.vector.tensor_tensor(out=wa[:, :, :],
                                    in0=attsrc.to_broadcast([128, 4, 3]),
                                    in1=ck_sb[:, :, :], op=ALU.mult)
            # 4 transposes into one psum tile (sequential col groups), 1 copy
            ftp = psum.tile([128, 4 * F], DT.bfloat16, tag="sq", name="ftp")
            for lt in range(4):
                m = LT[lt]
                nc.tensor.transpose(
                    ftp[:m, F * lt:F * (lt + 1)],
                    feat_sb[(s, b)][0:F, LT_OFF[lt]:LT_OFF[lt] + m],
                    ident_bf[0:F, 0:F])
            fts = ft_pool.tile([128, 4 * F], DT.bfloat16, tag="fts", name="fts")
            nc.vector.tensor_copy(fts[:, :], ftp[:, :])
            for lt in range(4):
                m = LT[lt]
                nc.tensor.matmul(out=S_ps[:, col:col + 3],
                                 lhsT=fts[:m, F * lt:F * (lt + 1)],
                                 rhs=wa[:m, lt, :], start=(lt == 0), stop=(lt == 3))

    S_sb = consts.tile([101, 6 * BLOC], DT.bfloat16, tag="Ssb", name="Ssb")
    nc.vector.tensor_copy(S_sb[0:F, :], S_ps[:, :])
    nc.sync.dma_start(out=S_sb[F:F + 1, :], in_=t["ones_row"].ap()[:, 0:6 * BLOC])

    am_ps = psum.tile([F, 2 * BLOC], DT.float32, tag="feat", name="feat")
    for si, s in enumerate("ui"):
        for b in range(BLOC):
            for k in range(3):
                nc.tensor.matmul(
                    out=am_ps[:, BLOC * si + b:BLOC * si + b + 1],
                    lhsT=wabs_sb[s][:, k, :],
                    rhs=S_sb[:, 3 * (BLOC * si + b) + k:3 * (BLOC * si + b) + k + 1],
                    start=(k == 0), stop=(k == 2))
    am_sb = sm_pool.tile([F, 2 * BLOC], DT.bfloat16, tag="am_sb", name="am_sb")
    nc.vector.tensor_copy(am_sb[:, :], am_ps[:, :])

    for si, (s, oname) in enumerate((("u", "out_use"), ("i", "out_item"))):
        fc_ps = psum.tile([ID, BLOC], DT.float32, tag="u", name="u")
        nc.tensor.matmul(out=fc_ps[:, :], lhsT=wfc_sb[s][:, :],
                         rhs=am_sb[:, BLOC * si:BLOC * (si + 1)],
                         start=True, stop=True)
        fcr = sm_pool.tile([ID, BLOC], DT.float32, tag="fcr", name="fcr")
        nc.scalar.activation(fcr[:, :], fc_ps[:, :], AF.Relu,
                             bias=bfc_sb[s][:, :])
        fct = psum.tile([BLOC, ID], DT.float32, tag="u", name="u")
        nc.tensor.transpose(fct[:, :], fcr[:, :], ident_f32[:ID, :ID])
        osb = sm_pool.tile([BLOC, 2 * ID], DT.float32, tag=f"osb{s}", name=f"osb{s}")
        nc.vector.tensor_copy(osb[:, 0:ID], fct[:, :])
        nc.gpsimd.indirect_dma_start(
            out=osb[:, ID:2 * ID], out_offset=None,
            in_=t[f"idemb_{s}"].ap(),
            in_offset=bass.IndirectOffsetOnAxis(ap=idid_sb[s][:, 0:1], axis=0))
        nc.sync.dma_start(out=t[oname].ap(), in_=osb[:, :])

    ctx.close()


# ======================= host side =======================

_PROG = None


def _get_prog():
    global _PROG
    if _PROG is None:
        _PROG = build_program()
    return _PROG


def _bf16_table(tab):
    out = np.zeros((V + 1, DPAD), dtype=BF16)
    out[:V, :D] = np.asarray(tab, dtype=np.float32)
    return out


def _gather_idx(doc):
    """doc: (BLOC, L) ids -> (128, 32*NG) int16 biased index tile"""
    stream = np.full((NG, GW), PADROW, dtype=np.int64)
    for b in range(BLOC):
        stream[b, 1:1 + L] = doc[b]
    biased = (stream - BIAS).astype(np.int16)
    arr = np.zeros((128, 32 * NG), dtype=np.int16)
    for g in range(NG):
        blk = biased[g].reshape(32, 16).T  # idx i -> [i%16, i//16]
        for r in range(8):
            arr[16 * r:16 * (r + 1), 32 * g:32 * (g + 1)] = blk
    return arr


def _window_counts():
    c = np.zeros((3, L), dtype=np.float64)
    for k in range(3):
        for lp in range(k, k + L - 2):
            for d2 in (-1, 0, 1):
                ll = lp + d2
                if 0 <= ll < L:
                    c[k, ll] += 1
    return c


def _prep_weights(inp):
    w = {}
    w3 = np.zeros((DPAD, 3), dtype=np.float32)
    w3[:D, :] = np.asarray(inp["word_cnn_w"][0, 0]).astype(np.float32).T
    w["w3"] = np.ascontiguousarray(w3.reshape(NCH, 128, 3).transpose(1, 0, 2)).astype(BF16)
    w["wcb"] = np.asarray(inp["word_cnn_b"]).astype(np.float32).reshape(1, 1)

    for s, key in (("u", "user"), ("i", "item")):
        dw = np.asarray(inp[f"{key}_doc_cnn_w"]).astype(np.float32)  # (F,1,3,D)
        arr = np.zeros((128, NCH, 3, F), dtype=BF16)
        for tp in range(3):
            pad = np.zeros((DPAD, F), dtype=np.float32)
            pad[:D] = dw[:, 0, tp, :].T
            arr[:, :, tp, :] = pad.reshape(NCH, 128, F).transpose(1, 0, 2)
        w[f"wd_{s}"] = arr
        dcb = np.asarray(inp[f"{key}_doc_cnn_b"]).astype(np.float32)
        w[f"dcb_{s}"] = (dcb * (-2.0 if s == "u" else 1.0)).reshape(F, 1)

        aw = np.asarray(inp[f"{key}_abs_cnn_w"]).astype(np.float32)  # (F,1,3,F)
        ab = np.asarray(inp[f"{key}_abs_cnn_b"]).astype(np.float32)
        scale = (1.0 / (L - 2)) * (-0.5 if s == "u" else 1.0)
        warr = np.zeros((101, 3, F), dtype=BF16)
        for k in range(3):
            warr[:F, k, :] = (aw[:, 0, k, :] * scale).T
        warr[F, 0, :] = ab
        w[f"wabs_{s}"] = warr

        w[f"wfc_{s}"] = np.asarray(inp[f"{key}_fc_w"]).astype(np.float32).T.astype(BF16)
        w[f"bfc_{s}"] = np.asarray(inp[f"{key}_fc_b"]).astype(np.float32).reshape(ID, 1)

    cw = _window_counts()
    ckt = np.zeros((128, 4, 3), dtype=BF16)
    for lt in range(4):
        m = LT[lt]
        ckt[:m, lt, :] = cw[:, LT_OFF[lt]:LT_OFF[lt] + m].T
    w["ck"] = ckt
    return w


def prepare_in_maps(inputs):
    w = _prep_weights(inputs)
    tab_u = _bf16_table(inputs["user_word_emb"])
    tab_i = _bf16_table(inputs["item_word_emb"])
    user_doc = np.asarray(inputs["user_doc"]).astype(np.int64)
    item_doc = np.asarray(inputs["item_doc"]).astype(np.int64)
    uids = np.asarray(inputs["uids"]).astype(np.int64)
    iids = np.asarray(inputs["iids"]).astype(np.int64)
    uid_emb = np.asarray(inputs["uid_emb"]).astype(np.float32)
    iid_emb = np.asarray(inputs["iid_emb"]).astype(np.float32)

    in_maps = []
    for c in range(NCORE):
        sl = slice(BLOC * c, BLOC * (c + 1))
        in_maps.append({
            "tab_u": tab_u, "tab_i": tab_i,
            "idx_u": _gather_idx(user_doc[sl]),
            "idx_i": _gather_idx(item_doc[sl]),
            "w3": w["w3"], "wcb": w["wcb"], "ck": w["ck"],
            "ones_row": np.ones((1, L), dtype=BF16),
            "wd_u": w["wd_u"], "wd_i": w["wd_i"],
            "dcb_u": w["dcb_u"], "dcb_i": w["dcb_i"],
            "wabs_u": w["wabs_u"], "wabs_i": w["wabs_i"],
            "wfc_u": w["wfc_u"], "wfc_i": w["wfc_i"],
            "bfc_u": w["bfc_u"], "bfc_i": w["bfc_i"],
            # crossed on purpose: use_fea carries iid_emb, item_fea uid_emb
            "idemb_u": iid_emb, "idemb_i": uid_emb,
            "idids_u": iids[sl].astype(np.int32).reshape(BLOC, 1),
            "idids_i": uids[sl].astype(np.int32).reshape(BLOC, 1),
        })
    return in_maps


def assemble_outputs(res):
    use = np.concatenate([np.asarray(res.results[c]["out_use"]) for c in range(NCORE)])
    item = np.concatenate([np.asarray(res.results[c]["out_item"]) for c in range(NCORE)])
    return (use.reshape(B, 2, ID).astype(np.float32),
            item.reshape(B, 2, ID).astype(np.float32))


def kernel(**inputs):
    nc = _get_prog()
    in_maps = prepare_in_maps(inputs)
    res = bass_utils.run_bass_kernel_spmd(nc, in_maps, core_ids=list(range(NCORE)))
    return assemble_outputs(res)

